# revision 6
# baseline (speedup 1.0000x reference)
"""GCN (2-layer mix-pass + projection MLP) on 8 Trainium2 NeuronCores.

Sharding: targets split contiguously across 8 cores (12544 nodes each,
N padded to 100352). Host packs, per core, "degree-slot" message arrays
(slot d = feature rows of the d-th in-edge source for every target,
targets sorted by in-degree so slots are prefix slices). The device
accumulates the slots (segment-sum), applies the node GEMMs, scaling,
relu / elu and log-softmax. Between the two graph layers the hidden
state returns to the host, which regathers messages for layer 2.
"""
import sys
sys.path.insert(0, '/opt/trn_rl_repo')
import numpy as np

N = 100_000
NPAD = 100_352          # 784 blocks of 128
NC = 8
SHARD = NPAD // NC      # 12544
D = 128
DOUT = 64
TILE = 128
NTILES = SHARD // TILE  # 98
STREAM = 8192           # msg stream SBUF tile width (columns)

_cache = {}
_last_hw_ns = None


def _build_programs(tot_pad, seg_plan, n_layers_meta):
    """Build + compile the two device programs. seg_plan: list of
    (acc_lo, acc_hi, st_lo) add segments in stream order."""
    import concourse.bass as bass
    import concourse.tile as tile
    from concourse import bacc, mybir

    fp16 = mybir.dt.float16
    f32 = mybir.dt.float32

    def common_inputs(nc):
        msgs = nc.dram_tensor("msgs", [128, tot_pad], fp16, kind="ExternalInput").ap()
        wT = nc.dram_tensor("wT", [128, D], fp16, kind="ExternalInput").ap()
        dinv_col = nc.dram_tensor("dinv_col", [128, NTILES], f32, kind="ExternalInput").ap()
        b_row = nc.dram_tensor("b_row", [128, D], f32, kind="ExternalInput").ap()
        return msgs, wT, dinv_col, b_row

    def emit_agg(nc, tc, ctx, msgs, acc):
        """Stream msgs from DRAM, accumulate slots into acc [128, SHARD] f32."""
        nc.vector.memset(acc[:], 0.0)
        import contextlib
        pool = ctx.enter_context(tc.tile_pool(name="stream", bufs=3))
        n_stream = (tot_pad + STREAM - 1) // STREAM
        cur = 0  # current stream tile index loaded
        tiles = {}
        for (alo, ahi, slo) in seg_plan:
            ti = slo // STREAM
            if ti not in tiles:
                w = min(STREAM, tot_pad - ti * STREAM)
                t = pool.tile([128, w], fp16, tag="ms")
                nc.sync.dma_start(t[:], msgs[:, ti * STREAM: ti * STREAM + w])
                tiles = {ti: t}  # keep only latest; pool recycles bufs
            t = tiles[ti]
            off = slo - ti * STREAM
            n = ahi - alo
            nc.vector.tensor_tensor(
                out=acc[:, alo:ahi], in0=acc[:, alo:ahi],
                in1=t[:, off:off + n], op=mybir.AluOpType.add)

    # ---- program 1 (layer 1): agg -> W1 -> relu/scale -> z' ----
    from contextlib import ExitStack
    nc1 = bacc.Bacc("TRN2", target_bir_lowering=False, debug=False, num_devices=NC)
    msgs, w1T, dinv_col, b1_row = common_inputs(nc1)
    zout = nc1.dram_tensor("zout", [SHARD, D], fp16, kind="ExternalOutput").ap()
    with tile.TileContext(nc1) as tc:
        with ExitStack() as ctx:
            import concourse.mybir as mybir_  # noqa
            accp = ctx.enter_context(tc.tile_pool(name="acc", bufs=1))
            cons = ctx.enter_context(tc.tile_pool(name="cons", bufs=1))
            work = ctx.enter_context(tc.tile_pool(name="work", bufs=6))
            psum = ctx.enter_context(tc.tile_pool(name="ps", bufs=4, space="PSUM"))
            acc = accp.tile([128, SHARD], f32)
            w1T_t = cons.tile([128, D], fp16)
            dinv_t = cons.tile([128, NTILES], f32)
            b1_t = cons.tile([128, D], f32)
            nc1.sync.dma_start(w1T_t[:], w1T[:])
            nc1.sync.dma_start(dinv_t[:], dinv_col[:])
            nc1.sync.dma_start(b1_t[:], b1_row[:])
            emit_agg(nc1, tc, ctx, msgs, acc)
            for i in range(NTILES):
                a16 = work.tile([128, TILE], fp16, tag="a16")
                nc1.vector.tensor_copy(a16[:], acc[:, i * TILE:(i + 1) * TILE])
                ps = psum.tile([TILE, D], f32, tag="mm")
                nc1.tensor.matmul(ps[:], lhsT=a16[:], rhs=w1T_t[:], start=True, stop=True)
                dc = dinv_t[:, i:i + 1]
                t1 = work.tile([TILE, D], f32, tag="t1")
                nc1.vector.tensor_scalar(out=t1[:], in0=ps[:], scalar1=dc,
                                         scalar2=None, op0=mybir.AluOpType.mult)
                t2 = work.tile([TILE, D], f32, tag="t2")
                nc1.vector.tensor_tensor(out=t2[:], in0=t1[:],
                                         in1=b1_t[:],
                                         op=mybir.AluOpType.add)
                zt = work.tile([TILE, D], fp16, tag="zt")
                nc1.vector.tensor_scalar(out=zt[:], in0=t2[:], scalar1=0.0,
                                         scalar2=dc, op0=mybir.AluOpType.max,
                                         op1=mybir.AluOpType.mult)
                nc1.sync.dma_start(zout[i * TILE:(i + 1) * TILE, :], zt[:])
    nc1.compile()

    # ---- program 2 (layer 2 + projection) ----
    nc2 = bacc.Bacc("TRN2", target_bir_lowering=False, debug=False, num_devices=NC)
    msgs2, w2T, dinv_col2, b2_row = common_inputs(nc2)
    b2_colt = nc2.dram_tensor("b2_col", [D, 1], f32, kind="ExternalInput").ap()
    dinv_rep = nc2.dram_tensor("dinv_rep", [128, SHARD], f32, kind="ExternalInput").ap()
    fcw1T = nc2.dram_tensor("fcw1T", [128, D], fp16, kind="ExternalInput").ap()
    fcw2T = nc2.dram_tensor("fcw2T", [128, DOUT], fp16, kind="ExternalInput").ap()
    fcb1_col = nc2.dram_tensor("fcb1_col", [128, 1], f32, kind="ExternalInput").ap()
    fcb2_row = nc2.dram_tensor("fcb2_row", [128, DOUT], f32, kind="ExternalInput").ap()
    zs_out = nc2.dram_tensor("zs_out", [128, SHARD], f32, kind="ExternalOutput").ap()
    res_out = nc2.dram_tensor("res_out", [SHARD, DOUT], f32, kind="ExternalOutput").ap()
    with tile.TileContext(nc2) as tc:
        with ExitStack() as ctx:
            accp = ctx.enter_context(tc.tile_pool(name="acc", bufs=1))
            cons = ctx.enter_context(tc.tile_pool(name="cons", bufs=1))
            drep = ctx.enter_context(tc.tile_pool(name="drep", bufs=1))
            work = ctx.enter_context(tc.tile_pool(name="work", bufs=6))
            psum = ctx.enter_context(tc.tile_pool(name="ps", bufs=2, space="PSUM"))
            psum2 = ctx.enter_context(tc.tile_pool(name="ps2", bufs=2, space="PSUM"))
            acc = accp.tile([128, SHARD], f32)
            w2T_t = cons.tile([128, D], fp16)
            fcw1T_t = cons.tile([128, D], fp16)
            fcw2T_t = cons.tile([128, DOUT], fp16)
            fcb1_t = cons.tile([128, 1], f32)
            fcb2_t = cons.tile([128, DOUT], f32)
            b2_t = cons.tile([D, 1], f32)
            dinv_t = cons.tile([128, NTILES], f32)
            drt = drep.tile([128, SHARD], f32)
            nc2.sync.dma_start(w2T_t[:], w2T[:])
            nc2.sync.dma_start(fcw1T_t[:], fcw1T[:])
            nc2.sync.dma_start(fcw2T_t[:], fcw2T[:])
            nc2.sync.dma_start(fcb1_t[:], fcb1_col[:])
            nc2.sync.dma_start(fcb2_t[:], fcb2_row[:])
            nc2.sync.dma_start(b2_t[:], b2_colt[:])
            nc2.sync.dma_start(dinv_t[:], dinv_col2[:])
            nc2.sync.dma_start(drt[:], dinv_rep[:])
            emit_agg(nc2, tc, ctx, msgs2, acc)
            for i in range(NTILES):
                sl = slice(i * TILE, (i + 1) * TILE)
                a16 = work.tile([128, TILE], fp16, tag="a16")
                nc2.vector.tensor_copy(a16[:], acc[:, sl])
                # zs feature-major: P = W2 @ AGG  -> [dh, t]
                psA = psum.tile([D, TILE], f32, tag="A")
                nc2.tensor.matmul(psA[:], lhsT=w2T_t[:], rhs=a16[:], start=True, stop=True)
                zs1 = work.tile([D, TILE], f32, tag="zs1")
                nc2.vector.tensor_tensor(out=zs1[:], in0=psA[:], in1=drt[:, sl],
                                         op=mybir.AluOpType.mult)
                zs = work.tile([D, TILE], f32, tag="zs")
                nc2.vector.tensor_scalar(out=zs[:], in0=zs1[:], scalar1=b2_t[:],
                                         scalar2=None, op0=mybir.AluOpType.add)
                nc2.sync.dma_start(zs_out[:, sl], zs[:])
                zs16 = work.tile([D, TILE], fp16, tag="zs16")
                nc2.vector.tensor_copy(zs16[:], zs[:])
                # h = elu(fcW1 @ zs + fcb1)   [dp, t]
                psB = psum.tile([D, TILE], f32, tag="B")
                nc2.tensor.matmul(psB[:], lhsT=fcw1T_t[:], rhs=zs16[:], start=True, stop=True)
                m = work.tile([D, TILE], fp16, tag="m")
                nc2.vector.tensor_scalar(out=m[:], in0=psB[:], scalar1=fcb1_t[:],
                                         scalar2=0.0, op0=mybir.AluOpType.add,
                                         op1=mybir.AluOpType.min)
                e = work.tile([D, TILE], f32, tag="e")
                nc2.scalar.activation(e[:], m[:], mybir.ActivationFunctionType.Exp)
                r = work.tile([D, TILE], f32, tag="r")
                nc2.vector.tensor_scalar(out=r[:], in0=psB[:], scalar1=fcb1_t[:],
                                         scalar2=0.0, op0=mybir.AluOpType.add,
                                         op1=mybir.AluOpType.max)
                h1 = work.tile([D, TILE], f32, tag="h1")
                nc2.vector.tensor_tensor(out=h1[:], in0=r[:], in1=e[:],
                                         op=mybir.AluOpType.add)
                h16 = work.tile([D, TILE], fp16, tag="h16")
                nc2.vector.tensor_scalar(out=h16[:], in0=h1[:], scalar1=-1.0,
                                         scalar2=None, op0=mybir.AluOpType.add)
                # o node-major: [t, 64] = h.T @ fcW2.T
                psC = psum2.tile([TILE, DOUT], f32, tag="C")
                nc2.tensor.matmul(psC[:], lhsT=h16[:], rhs=fcw2T_t[:], start=True, stop=True)
                ob = work.tile([TILE, DOUT], f32, tag="ob")
                nc2.vector.tensor_tensor(out=ob[:], in0=psC[:],
                                         in1=fcb2_t[:],
                                         op=mybir.AluOpType.add)
                mx = work.tile([TILE, 1], f32, tag="mx")
                nc2.vector.tensor_reduce(out=mx[:], in_=ob[:], axis=mybir.AxisListType.X,
                                         op=mybir.AluOpType.max)
                sh = work.tile([TILE, DOUT], f32, tag="sh")
                nc2.vector.tensor_scalar(out=sh[:], in0=ob[:], scalar1=mx[:],
                                         scalar2=None, op0=mybir.AluOpType.subtract)
                ex = work.tile([TILE, DOUT], f32, tag="ex")
                nc2.scalar.activation(ex[:], sh[:], mybir.ActivationFunctionType.Exp)
                sm = work.tile([TILE, 1], f32, tag="sm")
                nc2.vector.tensor_reduce(out=sm[:], in_=ex[:], axis=mybir.AxisListType.X,
                                         op=mybir.AluOpType.add)
                ln = work.tile([TILE, 1], f32, tag="ln")
                nc2.scalar.activation(ln[:], sm[:], mybir.ActivationFunctionType.Ln)
                rs = work.tile([TILE, DOUT], f32, tag="rs")
                nc2.vector.tensor_scalar(out=rs[:], in0=sh[:], scalar1=ln[:],
                                         scalar2=None, op0=mybir.AluOpType.subtract)
                nc2.sync.dma_start(res_out[sl, :], rs[:])
    nc2.compile()
    return nc1, nc2


def kernel(x, edge_index, W1, b1, W2, b2, fcW1, fcb1, fcW2, fcb2):
    x = np.asarray(x, dtype=np.float32)
    b1 = np.asarray(b1, dtype=np.float32); b2 = np.asarray(b2, dtype=np.float32)
    fcb1 = np.asarray(fcb1, dtype=np.float32); fcb2 = np.asarray(fcb2, dtype=np.float32)
    W1 = np.asarray(W1); W2 = np.asarray(W2); fcW1 = np.asarray(fcW1); fcW2 = np.asarray(fcW2)
    ei = np.asarray(edge_index).astype(np.int64)
    row, col = ei[0], ei[1]
    # self loops
    row = np.concatenate([row, np.arange(N, dtype=np.int64)])
    col = np.concatenate([col, np.arange(N, dtype=np.int64)])
    deg = np.bincount(col, minlength=NPAD).astype(np.float32)  # includes self loop
    dinv = np.zeros(NPAD, np.float32)
    nz = deg > 0
    dinv[nz] = 1.0 / np.sqrt(deg[nz])

    # per-core target shards; sort targets by in-degree desc within shard
    core_of = col // SHARD
    order = np.argsort(col, kind='stable')
    row_s, col_s = row[order], col[order]
    # build per-core slot structure
    per_core = []
    for c in range(NC):
        lo, hi = c * SHARD, (c + 1) * SHARD
        tdeg = deg[lo:hi].astype(np.int64)
        perm = np.argsort(-tdeg, kind='stable')      # sorted target order (local)
        inv_perm = np.argsort(perm, kind='stable')
        sdeg = tdeg[perm]
        # CSR of in-edges for this shard (col-sorted stream)
        mask = (col_s >= lo) & (col_s < hi)
        r_sh = row_s[mask]
        c_sh = col_s[mask] - lo
        starts = np.zeros(SHARD + 1, np.int64)
        np.cumsum(np.bincount(c_sh, minlength=SHARD), out=starts[1:])
        per_core.append(dict(perm=perm, inv_perm=inv_perm, sdeg=sdeg,
                             r_sh=r_sh, starts=starts, lo=lo))
    dmax = int(max(pc['sdeg'][0] if len(pc['sdeg']) else 0 for pc in per_core))
    # slot widths: per-d max over cores, rounded up a little for alignment
    n_d = np.zeros(dmax, np.int64)
    for d in range(dmax):
        n_d[d] = max(int((pc['sdeg'] >= d + 1).sum()) for pc in per_core)
    n_d = np.maximum(n_d, 1)
    offs = np.zeros(dmax + 1, np.int64)
    np.cumsum(n_d, out=offs[1:])
    tot = int(offs[-1])
    tot_pad = ((tot + STREAM - 1) // STREAM) * STREAM
    # segment plan: per slot d, split [0, n_d) at STREAM boundaries of (offs[d]+j)
    seg_plan = []
    for d in range(dmax):
        j = 0
        while j < n_d[d]:
            st = offs[d] + j
            room = STREAM - (st % STREAM)
            n = min(room, n_d[d] - j)
            seg_plan.append((j, j + n, int(st)))
            j += n
    key = (tot_pad, tuple(seg_plan[:8]), len(seg_plan))
    if key not in _cache:
        _cache[key] = _build_programs(tot_pad, seg_plan, None)
    nc1, nc2 = _cache[key]

    # per-core gather index lists (same for both layers)
    gidx = []
    for pc in per_core:
        gl = np.zeros(tot_pad, np.int64)  # default 0; msgs default 0 via zero rows? no:
        valid = np.zeros(tot_pad, bool)
        for d in range(dmax):
            nvalid = int((pc['sdeg'] >= d + 1).sum())
            tloc = pc['perm'][:nvalid]                      # local sorted targets with deg>d
            srcs = pc['r_sh'][pc['starts'][tloc] + d]
            gl[offs[d]:offs[d] + nvalid] = srcs
            valid[offs[d]:offs[d] + nvalid] = True
        gidx.append((gl, valid))

    def build_msgs(feat_nm):  # feat_nm [NPAD, D] fp16 node-major
        out = []
        for gl, valid in gidx:
            m = feat_nm[gl]              # [tot_pad, D]
            m[~valid] = 0
            out.append(np.ascontiguousarray(m.T))  # [128, tot_pad]
        return out

    from concourse.bass_utils import run_bass_kernel_spmd
    import os
    _trace = bool(int(os.environ.get("KERNEL_TRACE", "0")))

    xs = np.zeros((NPAD, D), np.float16)
    xs[:N] = (x * dinv[:N, None]).astype(np.float16)
    msgs1 = build_msgs(xs)
    f16 = np.float16
    w1T = np.ascontiguousarray(W1.astype(f16).T)     # [din, dh]
    w2T = np.ascontiguousarray(W2.astype(f16).T)
    fcw1T = np.ascontiguousarray(fcW1.astype(f16).T)
    fcw2T = np.ascontiguousarray(fcW2.astype(f16).T)

    in1, dinv_cols = [], []
    for c, pc in enumerate(per_core):
        dl = dinv[pc['lo']:pc['lo'] + SHARD][pc['perm']]  # sorted order
        dinv_cols.append(dl)
        in1.append(dict(msgs=msgs1[c], wT=w1T,
                        dinv_col=np.ascontiguousarray(dl.reshape(NTILES, TILE).T.astype(np.float32)),
                        b_row=np.ascontiguousarray(np.broadcast_to(b1.reshape(1, D), (128, D))).astype(np.float32)))
    kw1 = dict(trace=True, tmpdir="/tmp/trace_p1") if _trace else {}
    r1 = run_bass_kernel_spmd(nc1, in1, list(range(NC)), **kw1)
    # assemble z' full (node-major, original order)
    zp = np.zeros((NPAD, D), np.float16)
    for c, pc in enumerate(per_core):
        zp[pc['lo']:pc['lo'] + SHARD] = r1.results[c]["zout"][pc['inv_perm']]
    msgs2 = build_msgs(zp)
    in2 = []
    for c, pc in enumerate(per_core):
        dl = dinv_cols[c]
        in2.append(dict(msgs=msgs2[c], wT=w2T,
                        dinv_col=in1[c]['dinv_col'],
                        b_row=np.zeros((128, D), np.float32),
                        b2_col=b2.reshape(D, 1).astype(np.float32),
                        dinv_rep=np.ascontiguousarray(
                            np.broadcast_to(dl[None, :], (128, SHARD))).astype(np.float32),
                        fcw1T=fcw1T, fcw2T=fcw2T,
                        fcb1_col=fcb1.reshape(128, 1).astype(np.float32),
                        fcb2_row=np.ascontiguousarray(np.broadcast_to(fcb2.reshape(1, DOUT), (128, DOUT))).astype(np.float32)))
    kw2 = dict(trace=True, tmpdir="/tmp/trace_p2") if _trace else {}
    r2 = run_bass_kernel_spmd(nc2, in2, list(range(NC)), **kw2)
    global _last_hw_ns
    if _trace and r1.exec_time_ns and r2.exec_time_ns:
        _last_hw_ns = int(r1.exec_time_ns) + int(r2.exec_time_ns)
    zs = np.zeros((NPAD, D), np.float32)
    res = np.zeros((NPAD, DOUT), np.float32)
    for c, pc in enumerate(per_core):
        zs[pc['lo']:pc['lo'] + SHARD] = r2.results[c]["zs_out"].T[pc['inv_perm']]
        res[pc['lo']:pc['lo'] + SHARD] = r2.results[c]["res_out"][pc['inv_perm']]
    return zs[:N], res[:N]
